# revision 14
# baseline (speedup 1.0000x reference)
"""Trainium2 Bass kernel for single-head (H=1) masked self-attention over
128 independent graphs of 512 nodes (d_model = 512).

Math (per graph b, X = data rows of b, all [512, 512]):
    S  = (1/sqrt(512)) * X Wq^T Wk X^T        (key-mask -> -inf cols)
    A  = softmax(S, axis=-1), masked cols zeroed
    out = A V Wo^T + b,  V = X Wv^T

Device strategy (data-parallel over batch, 16 graphs per NeuronCore):
  * Host folds Wq/Wk into one matrix Wqk = norm * Wq^T @ Wk (512x512), so
    scores take 2 matmuls instead of 3.
  * Host supplies X^T per graph ([i, g]), so no on-device transposes are
    needed anywhere:
        P1T  = matmul(lhsT=Wqk,  rhs=XT)      # (X Wqk)^T      [i', q]
        ST   = matmul(lhsT=XT,   rhs=P1T)     # S^T            [k,  q]
        Pexp = exp(ST + mask_bias[k])         # ACT, bias -30000 on masked k
        V    = matmul(lhsT=XT,   rhs=WvT)     # X Wv^T         [g,  dv]
        HT   = matmul(lhsT=V,    rhs=Pexp)    # (A_unnorm V)^T [dv, q]
        den  = ones^T @ Pexp                  # softmax denominators [1, q]
        out  = matmul(lhsT=HT,   rhs=WoT) * (1/den)[q] + bias  # [q, e]
    Softmax runs in S^T layout (keys on partitions) so the mask bias is a
    per-partition ACT bias and exp(-30000) = 0 reproduces the exact masked
    zeros of the reference.
  * The [1,512] denominator row is transposed to [128,4] with 4 rank-1
    PE matmuls so normalization is a per-partition scalar multiply.
"""

import math

import numpy as np

N_CORES = 8
B = 128          # graphs
G = 512          # nodes per graph
D = 512          # model dim
BPC = B // N_CORES   # graphs per core
P = 128          # SBUF partitions
NC_ = D // P     # 4 chunks of 128

MASK_BIAS = -30000.0

_CACHE: dict = {}


def _build_nc(mm_fast: bool):
    import concourse.tile as tile
    from concourse import bacc, mybir

    f32 = mybir.dt.float32
    # float32r: fp32 bits, single-pass PE mode (1 cyc/row at N>=512 vs 4 for
    # exact fp32). All matmul operands must be produced as float32r, so the
    # tiles (and the DRAM tensors they are DMA'd from) use this dtype.
    mdt = mybir.dt.float32r if mm_fast else mybir.dt.float32

    nc = bacc.Bacc("TRN2", target_bir_lowering=False, debug=False,
                   num_devices=N_CORES)

    xt_ap = nc.dram_tensor("xt", [BPC, D, G], mdt, kind="ExternalInput").ap()
    wqk_ap = nc.dram_tensor("wqk", [D, D], mdt, kind="ExternalInput").ap()
    wvt_ap = nc.dram_tensor("wvt", [D, D], mdt, kind="ExternalInput").ap()
    wot_ap = nc.dram_tensor("wot", [D, D], mdt, kind="ExternalInput").ap()
    biasb_ap = nc.dram_tensor("biasb", [P, D], f32, kind="ExternalInput").ap()
    maskb_ap = nc.dram_tensor("maskb", [P, BPC, NC_], f32,
                              kind="ExternalInput").ap()
    y_ap = nc.dram_tensor("y", [BPC * G, D], f32, kind="ExternalOutput").ap()

    with tile.TileContext(nc) as tc:
        with (
            tc.tile_pool(name="const", bufs=1) as const_pool,
            tc.tile_pool(name="xt", bufs=2) as xt_pool,
            tc.tile_pool(name="p1t", bufs=2) as p1t_pool,
            tc.tile_pool(name="pexp", bufs=2) as pexp_pool,
            tc.tile_pool(name="v", bufs=2) as v_pool,
            tc.tile_pool(name="h", bufs=2) as h_pool,
            tc.tile_pool(name="outp", bufs=2) as out_pool,
            tc.tile_pool(name="small", bufs=2) as small_pool,
            tc.tile_pool(name="psum", bufs=7, space="PSUM") as psum_pool,
            tc.tile_pool(name="psum_dcol", bufs=1, space="PSUM") as psd_pool,
        ):
            # --- one-time constants ---
            # Only wqk / maskb (and xt[0], loaded in the loop) gate the first
            # matmuls; the other constant DMAs are emitted inside batch 0's
            # body so they don't delay the PE start on the FIFO DMA queue.
            wqk_sb = const_pool.tile([P, NC_, D], mdt, tag="wqk")
            nc.sync.dma_start(wqk_sb[:],
                              wqk_ap.rearrange("(po pi) j -> pi po j", pi=P))
            maskb_sb = const_pool.tile([P, BPC, NC_], f32, tag="maskb")
            nc.sync.dma_start(maskb_sb[:], maskb_ap[:])
            wvt_sb = const_pool.tile([P, NC_, D], mdt, tag="wvt")
            wot_sb = const_pool.tile([P, NC_, D], mdt, tag="wot")
            biasb_sb = const_pool.tile([P, D], f32, tag="biasb")
            # all-ones lhsT for the denominator reduction (f32r needs full
            # 128-column weights, so M=1 is padded to M=128: every output
            # row of the matmul is the same denominator row)
            ones_mat = const_pool.tile([P, P], mdt, tag="ones_mat")
            if mm_fast:
                # memset can't emit float32r directly; stage fp32 + cast-copy
                ones_f32 = const_pool.tile([P, P], f32, tag="ones_f32")
                nc.any.memset(ones_f32[:], 1.0)
                nc.vector.tensor_copy(ones_mat[:], ones_f32[:])
            else:
                nc.any.memset(ones_mat[:], 1.0)
            # rank-1 row->column transpose matmuls stay plain fp32 (walrus
            # rejects K=1 shapes in f32r mode)
            one_sc = const_pool.tile([1, 1], f32, tag="one_sc")
            nc.any.memset(one_sc[:], 1.0)

            for b in range(BPC):
                # load X^T for this graph: [128, 4, 512] (i on partitions)
                xt_sb = xt_pool.tile([P, NC_, G], mdt, tag="xt")
                nc.sync.dma_start(
                    xt_sb[:],
                    xt_ap[b].rearrange("(po pi) g -> pi po g", pi=P))

                # P1T[i', q] = Wqk^T X^T
                p1t_sb = p1t_pool.tile([P, NC_, G], mdt, tag="p1t")
                for mc in range(NC_):
                    ps = psum_pool.tile([P, G], f32, tag="big")
                    for kc in range(NC_):
                        nc.tensor.matmul(
                            ps[:],
                            wqk_sb[:, kc, mc * P:(mc + 1) * P],
                            xt_sb[:, kc, :],
                            start=(kc == 0), stop=(kc == NC_ - 1))
                    nc.vector.tensor_copy(p1t_sb[:, mc, :], ps[:])

                if b == 0:
                    nc.sync.dma_start(
                        wvt_sb[:],
                        wvt_ap.rearrange("(po pi) j -> pi po j", pi=P))

                # ST[k, q] = X P1 (scores transposed), then Pexp = exp(+bias)
                pexp_sb = pexp_pool.tile([P, NC_, G], mdt, tag="pexp")
                for mc in range(NC_):
                    ps = psum_pool.tile([P, G], f32, tag="big")
                    for kc in range(NC_):
                        nc.tensor.matmul(
                            ps[:],
                            xt_sb[:, kc, mc * P:(mc + 1) * P],
                            p1t_sb[:, kc, :],
                            start=(kc == 0), stop=(kc == NC_ - 1))
                    nc.scalar.activation(
                        pexp_sb[:, mc, :], ps[:],
                        mybir.ActivationFunctionType.Exp,
                        bias=maskb_sb[:, b, mc:mc + 1], scale=1.0)

                if b == 0:
                    nc.sync.dma_start(
                        wot_sb[:],
                        wot_ap.rearrange("(po pi) j -> pi po j", pi=P))
                    nc.sync.dma_start(biasb_sb[:], biasb_ap[:])

                # V[g, dv] = X Wv^T
                v_sb = v_pool.tile([P, NC_, D], mdt, tag="v")
                for mc in range(NC_):
                    ps = psum_pool.tile([P, D], f32, tag="big")
                    for kc in range(NC_):
                        nc.tensor.matmul(
                            ps[:],
                            xt_sb[:, kc, mc * P:(mc + 1) * P],
                            wvt_sb[:, kc, :],
                            start=(kc == 0), stop=(kc == NC_ - 1))
                    nc.vector.tensor_copy(v_sb[:, mc, :], ps[:])

                # denominators: ones^T Pexp -> [128, 512], every row = den[q]
                ps_row = psum_pool.tile([P, G], f32, tag="big")
                for kc in range(NC_):
                    nc.tensor.matmul(
                        ps_row[:],
                        ones_mat[:],
                        pexp_sb[:, kc, :],
                        start=(kc == 0), stop=(kc == NC_ - 1))
                row_sb = small_pool.tile([1, G], f32, tag="row_sb")
                nc.vector.tensor_copy(row_sb[:], ps_row[0:1, :])

                # transpose den row -> [128, 4] (4 rank-1 matmuls)
                ps_d = psd_pool.tile([P, NC_], f32, tag="dcol")
                for qc in range(NC_):
                    nc.tensor.matmul(
                        ps_d[:, qc:qc + 1],
                        row_sb[0:1, qc * P:(qc + 1) * P],
                        one_sc[:],
                        start=True, stop=True)
                recip_sb = small_pool.tile([P, NC_], f32, tag="recip")
                nc.vector.reciprocal(recip_sb[:], ps_d[:])

                # HT[dv, q] = (A_unnorm V)^T
                h_sb = h_pool.tile([P, NC_, G], mdt, tag="h")
                for mc in range(NC_):
                    ps = psum_pool.tile([P, G], f32, tag="big")
                    for kc in range(NC_):
                        nc.tensor.matmul(
                            ps[:],
                            v_sb[:, kc, mc * P:(mc + 1) * P],
                            pexp_sb[:, kc, :],
                            start=(kc == 0), stop=(kc == NC_ - 1))
                    nc.vector.tensor_copy(h_sb[:, mc, :], ps[:])

                # out[q, e] = HT^T WoT, normalized per-q, plus output bias
                out_sb = out_pool.tile([P, NC_, D], f32, tag="out")
                for mc in range(NC_):
                    ps = psum_pool.tile([P, D], f32, tag="big")
                    for kc in range(NC_):
                        nc.tensor.matmul(
                            ps[:],
                            h_sb[:, kc, mc * P:(mc + 1) * P],
                            wot_sb[:, kc, :],
                            start=(kc == 0), stop=(kc == NC_ - 1))
                    # scale on ACT (Copy is resident in every table set, so
                    # no table switching against the Exp ops), bias on DVE
                    nc.scalar.activation(
                        out_sb[:, mc, :], ps[:],
                        mybir.ActivationFunctionType.Copy,
                        scale=recip_sb[:, mc:mc + 1])
                    nc.vector.tensor_add(
                        out_sb[:, mc, :], out_sb[:, mc, :], biasb_sb[:])
                    # store each 128-row chunk as soon as it is ready
                    nc.sync.dma_start(
                        y_ap[b * G + mc * P:b * G + (mc + 1) * P],
                        out_sb[:, mc, :])

    nc.compile()
    return nc


def _get_nc(mm_fast: bool = True):
    key = ("nc", mm_fast)
    if key not in _CACHE:
        _CACHE[key] = _build_nc(mm_fast)
    return _CACHE[key]


def _host_prep(data, W_query, W_key, W_value, W_out_w, W_out_b, mask):
    data = np.asarray(data, dtype=np.float32)
    W_query = np.asarray(W_query, dtype=np.float32)
    W_key = np.asarray(W_key, dtype=np.float32)
    W_value = np.asarray(W_value, dtype=np.float32)
    W_out_w = np.asarray(W_out_w, dtype=np.float32)
    W_out_b = np.asarray(W_out_b, dtype=np.float32)
    mask = np.asarray(mask).astype(bool)

    norm = 1.0 / math.sqrt(D)
    wqk = (norm * (W_query.T.astype(np.float64) @ W_key.astype(np.float64))
           ).astype(np.float32)
    wvt = np.ascontiguousarray(W_value.T)
    wot = np.ascontiguousarray(W_out_w.T)
    biasb = np.broadcast_to(W_out_b, (P, D)).copy()
    # [128, B, 4]: maskb[p, b, kc] = bias for key k = kc*128 + p of graph b
    maskb_full = np.where(mask, np.float32(MASK_BIAS), np.float32(0.0)) \
        .reshape(B, NC_, P).transpose(2, 0, 1).copy()
    # X^T per graph: [B, i, g]
    dataT = data.reshape(B, G, D).transpose(0, 2, 1)

    in_maps = []
    for c in range(N_CORES):
        b0 = c * BPC
        in_maps.append({
            "xt": np.ascontiguousarray(dataT[b0:b0 + BPC]),
            "wqk": wqk,
            "wvt": wvt,
            "wot": wot,
            "biasb": biasb,
            "maskb": np.ascontiguousarray(maskb_full[:, b0:b0 + BPC, :]),
        })
    return in_maps


def kernel(data, W_query, W_key, W_value, W_out_w, W_out_b, mask,
           graph_size):
    from concourse.bass_utils import run_bass_kernel_spmd

    assert int(graph_size) == G
    in_maps = _host_prep(data, W_query, W_key, W_value, W_out_w, W_out_b,
                         mask)
    nc = _get_nc()
    res = run_bass_kernel_spmd(nc, in_maps, list(range(N_CORES)))
    y = np.concatenate([res.results[c]["y"] for c in range(N_CORES)], axis=0)
    return np.ascontiguousarray(y, dtype=np.float32)


# revision 15
# speedup vs baseline: 1.2794x; 1.2794x over previous
"""Trainium2 Bass kernel for single-head (H=1) masked self-attention over
128 independent graphs of 512 nodes (d_model = 512).

Math (per graph b, X = data rows of b, all [512, 512]):
    S  = (1/sqrt(512)) * X Wq^T Wk X^T        (key-mask -> -inf cols)
    A  = softmax(S, axis=-1), masked cols zeroed
    out = A V Wo^T + b,  V = X Wv^T

Device strategy (data-parallel over batch, 16 graphs per NeuronCore):
  * Host folds Wq/Wk into one matrix Wqk = norm * Wq^T @ Wk (512x512), so
    scores take 2 GEMMs instead of 3 (5 GEMMs/graph total).
  * Host supplies X^T per graph ([i, g]), so no on-device transposes are
    needed anywhere:
        P1T  = matmul(lhsT=Wqk,  rhs=XT)      # (X Wqk)^T      [i', q]
        ST   = matmul(lhsT=XT,   rhs=P1T)     # S^T            [k,  q]
        Pexp = exp(ST + mask_bias[k])         # ACT, bias -30000 on masked k
        V    = matmul(lhsT=XT,   rhs=WvT)     # X Wv^T         [g,  dv]
        den  = ones^T @ Pexp                  # softmax denominators
        HT   = matmul(lhsT=V,    rhs=Pexp) * (1/den)[q]   # A V normalized
        out  = matmul(lhsT=HT,   rhs=WoT) + bias          # [q, e]
    Softmax runs in S^T layout (keys on partitions) so the mask bias is a
    per-partition ACT bias and exp(-30000) = 0 reproduces the exact masked
    zeros of the reference.
  * All GEMMs run as float32r (single-pass PE mode, 4x the fp32 rate,
    ~2e-4 rel err). The denominator uses an all-ones [128,128] lhsT (f32r
    requires full 128-column weights), a fast-approx reciprocal on DVE and
    a GpSimd partition_broadcast so normalization fuses into the HT
    PSUM->SBUF evacuation.
"""

import math

import numpy as np

N_CORES = 8
B = 128          # graphs
G = 512          # nodes per graph
D = 512          # model dim
BPC = B // N_CORES   # graphs per core
P = 128          # SBUF partitions
NC_ = D // P     # 4 chunks of 128

MASK_BIAS = -30000.0

_CACHE: dict = {}


def _build_nc(mm_fast: bool):
    import concourse.tile as tile
    from concourse import bacc, mybir

    f32 = mybir.dt.float32
    mdt = mybir.dt.float32r if mm_fast else mybir.dt.float32

    nc = bacc.Bacc("TRN2", target_bir_lowering=False, debug=False,
                   num_devices=N_CORES)

    xt_ap = nc.dram_tensor("xt", [BPC, D, G], mdt, kind="ExternalInput").ap()
    wqk_ap = nc.dram_tensor("wqk", [D, D], mdt, kind="ExternalInput").ap()
    wvt_ap = nc.dram_tensor("wvt", [D, D], mdt, kind="ExternalInput").ap()
    wot_ap = nc.dram_tensor("wot", [D, D], mdt, kind="ExternalInput").ap()
    biasb_ap = nc.dram_tensor("biasb", [P, D], f32, kind="ExternalInput").ap()
    maskb_ap = nc.dram_tensor("maskb", [P, BPC, NC_], f32,
                              kind="ExternalInput").ap()
    y_ap = nc.dram_tensor("y", [BPC * G, D], f32, kind="ExternalOutput").ap()

    with tile.TileContext(nc) as tc:
        with (
            tc.tile_pool(name="const", bufs=1) as const_pool,
            tc.tile_pool(name="xt", bufs=2) as xt_pool,
            tc.tile_pool(name="p1t", bufs=2) as p1t_pool,
            tc.tile_pool(name="pexp", bufs=2) as pexp_pool,
            tc.tile_pool(name="v", bufs=2) as v_pool,
            tc.tile_pool(name="h", bufs=2) as h_pool,
            tc.tile_pool(name="outp", bufs=2) as out_pool,
            tc.tile_pool(name="small", bufs=2) as small_pool,
            tc.tile_pool(name="psum", bufs=8, space="PSUM") as psum_pool,
        ):
            # --- one-time constants ---
            # Only wqk / maskb (and xt[0], loaded in the loop) gate the first
            # matmuls; the other constant DMAs are emitted inside batch 0's
            # body so they don't delay the PE start on the FIFO DMA queue.
            wqk_sb = const_pool.tile([P, NC_, D], mdt, tag="wqk")
            for kc in range(NC_):
                nc.sync.dma_start(wqk_sb[:, kc, :],
                                  wqk_ap[kc * P:(kc + 1) * P, :])
            maskb_sb = const_pool.tile([P, BPC, NC_], f32, tag="maskb")
            nc.sync.dma_start(maskb_sb[:], maskb_ap[:])
            wvt_sb = const_pool.tile([P, NC_, D], mdt, tag="wvt")
            wot_sb = const_pool.tile([P, NC_, D], mdt, tag="wot")
            biasb_sb = const_pool.tile([P, D], f32, tag="biasb")
            # all-ones lhsT for the denominator reduction (f32r needs full
            # 128-column weights, so M=1 is padded to M=128: every output
            # row of the matmul is the same denominator row)
            ones_mat = const_pool.tile([P, P], mdt, tag="ones_mat")
            if mm_fast:
                # memset can't emit float32r directly; stage fp32 + cast-copy
                ones_f32 = const_pool.tile([P, P], f32, tag="ones_f32")
                nc.any.memset(ones_f32[:], 1.0)
                nc.vector.tensor_copy(ones_mat[:], ones_f32[:])
            else:
                nc.any.memset(ones_mat[:], 1.0)

            for b in range(BPC):
                # load X^T for this graph: [128, 4, 512] (i on partitions),
                # chunked so the first matmul only waits for chunk 0
                xt_sb = xt_pool.tile([P, NC_, G], mdt, tag="xt")
                for kc in range(NC_):
                    nc.sync.dma_start(xt_sb[:, kc, :],
                                      xt_ap[b, kc * P:(kc + 1) * P, :])

                # P1T[i', q] = Wqk^T X^T
                p1t_sb = p1t_pool.tile([P, NC_, G], mdt, tag="p1t")
                for mc in range(NC_):
                    ps = psum_pool.tile([P, G], f32, tag="big")
                    for kc in range(NC_):
                        nc.tensor.matmul(
                            ps[:],
                            wqk_sb[:, kc, mc * P:(mc + 1) * P],
                            xt_sb[:, kc, :],
                            start=(kc == 0), stop=(kc == NC_ - 1))
                    nc.vector.tensor_copy(p1t_sb[:, mc, :], ps[:])

                if b == 0:
                    for kc in range(NC_):
                        nc.sync.dma_start(wvt_sb[:, kc, :],
                                          wvt_ap[kc * P:(kc + 1) * P, :])

                # ST[k, q] = X P1 (scores transposed), then Pexp = exp(+bias)
                pexp_sb = pexp_pool.tile([P, NC_, G], mdt, tag="pexp")
                for mc in range(NC_):
                    ps = psum_pool.tile([P, G], f32, tag="big")
                    for kc in range(NC_):
                        nc.tensor.matmul(
                            ps[:],
                            xt_sb[:, kc, mc * P:(mc + 1) * P],
                            p1t_sb[:, kc, :],
                            start=(kc == 0), stop=(kc == NC_ - 1))
                    nc.scalar.activation(
                        pexp_sb[:, mc, :], ps[:],
                        mybir.ActivationFunctionType.Exp,
                        bias=maskb_sb[:, b, mc:mc + 1], scale=1.0)

                if b == 0:
                    for kc in range(NC_):
                        nc.sync.dma_start(wot_sb[:, kc, :],
                                          wot_ap[kc * P:(kc + 1) * P, :])
                    nc.sync.dma_start(biasb_sb[:], biasb_ap[:])

                # V[g, dv] = X Wv^T  (evacuated on ACT: Copy lives in every
                # activation table set, so no table switch against Exp)
                v_sb = v_pool.tile([P, NC_, D], mdt, tag="v")
                for mc in range(NC_):
                    ps = psum_pool.tile([P, D], f32, tag="big")
                    for kc in range(NC_):
                        nc.tensor.matmul(
                            ps[:],
                            xt_sb[:, kc, mc * P:(mc + 1) * P],
                            wvt_sb[:, kc, :],
                            start=(kc == 0), stop=(kc == NC_ - 1))
                    nc.scalar.activation(
                        v_sb[:, mc, :], ps[:],
                        mybir.ActivationFunctionType.Copy)

                # denominators: ones^T Pexp -> [128, 512], every row = den[q];
                # reciprocal of row 0, broadcast back to 128 partitions
                ps_row = psum_pool.tile([P, G], f32, tag="big")
                for kc in range(NC_):
                    nc.tensor.matmul(
                        ps_row[:],
                        ones_mat[:],
                        pexp_sb[:, kc, :],
                        start=(kc == 0), stop=(kc == NC_ - 1))
                recrow_sb = small_pool.tile([1, G], f32, tag="recrow")
                nc.vector.reciprocal_approx_fast(recrow_sb[:], ps_row[0:1, :])
                rb_sb = small_pool.tile([P, G], f32, tag="rb")
                nc.gpsimd.partition_broadcast(rb_sb[:], recrow_sb[:])

                # HT[dv, q] = (A V)^T: normalization fused into evacuation
                h_sb = h_pool.tile([P, NC_, G], mdt, tag="h")
                for mc in range(NC_):
                    ps = psum_pool.tile([P, G], f32, tag="big")
                    for kc in range(NC_):
                        nc.tensor.matmul(
                            ps[:],
                            v_sb[:, kc, mc * P:(mc + 1) * P],
                            pexp_sb[:, kc, :],
                            start=(kc == 0), stop=(kc == NC_ - 1))
                    nc.vector.tensor_mul(h_sb[:, mc, :], ps[:], rb_sb[:])

                # out[q, e] = HT^T WoT + bias
                out_sb = out_pool.tile([P, NC_, D], f32, tag="out")
                for mc in range(NC_):
                    ps = psum_pool.tile([P, D], f32, tag="big")
                    for kc in range(NC_):
                        nc.tensor.matmul(
                            ps[:],
                            h_sb[:, kc, mc * P:(mc + 1) * P],
                            wot_sb[:, kc, :],
                            start=(kc == 0), stop=(kc == NC_ - 1))
                    nc.vector.tensor_add(out_sb[:, mc, :], ps[:], biasb_sb[:])
                    # store each 128-row chunk as soon as it is ready
                    nc.sync.dma_start(
                        y_ap[b * G + mc * P:b * G + (mc + 1) * P],
                        out_sb[:, mc, :])

    nc.compile()
    return nc


def _get_nc(mm_fast: bool = True):
    key = ("nc", mm_fast)
    if key not in _CACHE:
        _CACHE[key] = _build_nc(mm_fast)
    return _CACHE[key]


def _host_prep(data, W_query, W_key, W_value, W_out_w, W_out_b, mask):
    data = np.asarray(data, dtype=np.float32)
    W_query = np.asarray(W_query, dtype=np.float32)
    W_key = np.asarray(W_key, dtype=np.float32)
    W_value = np.asarray(W_value, dtype=np.float32)
    W_out_w = np.asarray(W_out_w, dtype=np.float32)
    W_out_b = np.asarray(W_out_b, dtype=np.float32)
    mask = np.asarray(mask).astype(bool)

    norm = 1.0 / math.sqrt(D)
    wqk = (norm * (W_query.T.astype(np.float64) @ W_key.astype(np.float64))
           ).astype(np.float32)
    wvt = np.ascontiguousarray(W_value.T)
    wot = np.ascontiguousarray(W_out_w.T)
    biasb = np.broadcast_to(W_out_b, (P, D)).copy()
    # [128, B, 4]: maskb[p, b, kc] = bias for key k = kc*128 + p of graph b
    maskb_full = np.where(mask, np.float32(MASK_BIAS), np.float32(0.0)) \
        .reshape(B, NC_, P).transpose(2, 0, 1).copy()
    # X^T per graph: [B, i, g]
    dataT = data.reshape(B, G, D).transpose(0, 2, 1)

    in_maps = []
    for c in range(N_CORES):
        b0 = c * BPC
        in_maps.append({
            "xt": np.ascontiguousarray(dataT[b0:b0 + BPC]),
            "wqk": wqk,
            "wvt": wvt,
            "wot": wot,
            "biasb": biasb,
            "maskb": np.ascontiguousarray(maskb_full[:, b0:b0 + BPC, :]),
        })
    return in_maps


def kernel(data, W_query, W_key, W_value, W_out_w, W_out_b, mask,
           graph_size):
    from concourse.bass_utils import run_bass_kernel_spmd

    assert int(graph_size) == G
    in_maps = _host_prep(data, W_query, W_key, W_value, W_out_w, W_out_b,
                         mask)
    nc = _get_nc()
    res = run_bass_kernel_spmd(nc, in_maps, list(range(N_CORES)))
    y = np.concatenate([res.results[c]["y"] for c in range(N_CORES)], axis=0)
    return np.ascontiguousarray(y, dtype=np.float32)


# revision 16
# speedup vs baseline: 1.5261x; 1.1929x over previous
"""Trainium2 Bass kernel for single-head (H=1) masked self-attention over
128 independent graphs of 512 nodes (d_model = 512).

Math (per graph b, X = data rows of b, all [512, 512]):
    S  = (1/sqrt(512)) * X Wq^T Wk X^T        (key-mask -> -inf cols)
    A  = softmax(S, axis=-1), masked cols zeroed
    out = A V Wo^T + b,  V = X Wv^T

Device strategy (data-parallel over batch, 16 graphs per NeuronCore).
Two host-side weight foldings cut the per-graph GEMM count from 6 to 4
(the provable minimum for this module):
    Wqk = norm * Wq^T @ Wk          ->  S   = X Wqk X^T      (2 GEMMs)
    Wvo = (Wo @ Wv)^T               ->  out = A (X Wvo) + b  (2 GEMMs)
Host supplies X^T per graph ([i, g]) so no on-device transposes are needed:
    P1T  = matmul(lhsT=Wqk,  rhs=XT)      # (X Wqk)^T        [i', q]
    ST   = matmul(lhsT=XT,   rhs=P1T)     # S^T              [k,  q]
    Pexp = exp(ST + mask_bias[k])         # ACT, bias -30000 on masked k
    V2   = matmul(lhsT=XT,   rhs=Wvo)     # X Wvo            [g,  e]
    den  = ones^T @ Pexp                  # softmax denominators
    outT = matmul(lhsT=V2,   rhs=Pexp) * (1/den)[q] + bias[e]   # [e, q]
The kernel writes the output transposed ([e, q] per graph); the host
transposes it back (pure layout work, like the input X^T prep).

Softmax runs in S^T layout (keys on partitions) so the mask bias is a
per-partition ACT bias and exp(-30000) = 0 reproduces the exact masked
zeros of the reference. Normalization uses an all-ones [128,128] lhsT
matmul for the denominators (f32r requires full 128-column weights), a
fast-approx reciprocal on DVE, and a GpSimd partition_broadcast so the
divide fuses into the output PSUM->SBUF evacuation.

All GEMMs run as float32r: single-pass PE mode, 4x the exact-fp32 rate,
~2e-4 relative error end-to-end.
"""

import math

import numpy as np

N_CORES = 8
B = 128          # graphs
G = 512          # nodes per graph
D = 512          # model dim
BPC = B // N_CORES   # graphs per core
P = 128          # SBUF partitions
NC_ = D // P     # 4 chunks of 128

MASK_BIAS = -30000.0

_CACHE: dict = {}


def _build_nc(mm_fast: bool):
    import concourse.tile as tile
    from concourse import bacc, mybir

    f32 = mybir.dt.float32
    mdt = mybir.dt.float32r if mm_fast else mybir.dt.float32

    nc = bacc.Bacc("TRN2", target_bir_lowering=False, debug=False,
                   num_devices=N_CORES)

    xt_ap = nc.dram_tensor("xt", [BPC, D, G], mdt, kind="ExternalInput").ap()
    wqk_ap = nc.dram_tensor("wqk", [D, D], mdt, kind="ExternalInput").ap()
    wvo_ap = nc.dram_tensor("wvo", [D, D], mdt, kind="ExternalInput").ap()
    biasc_ap = nc.dram_tensor("biasc", [P, NC_], f32,
                              kind="ExternalInput").ap()
    maskb_ap = nc.dram_tensor("maskb", [P, BPC, NC_], f32,
                              kind="ExternalInput").ap()
    # output transposed: yt[b] = out[b].T  ([e, q] layout)
    yt_ap = nc.dram_tensor("yt", [BPC, D, G], f32, kind="ExternalOutput").ap()

    with tile.TileContext(nc) as tc:
        with (
            tc.tile_pool(name="const", bufs=1) as const_pool,
            tc.tile_pool(name="xt", bufs=2) as xt_pool,
            tc.tile_pool(name="p1t", bufs=2) as p1t_pool,
            tc.tile_pool(name="pexp", bufs=2) as pexp_pool,
            tc.tile_pool(name="v2", bufs=2) as v2_pool,
            tc.tile_pool(name="outp", bufs=2) as out_pool,
            tc.tile_pool(name="small", bufs=2) as small_pool,
            tc.tile_pool(name="psum", bufs=8, space="PSUM") as psum_pool,
        ):
            # --- one-time constants ---
            # Only wqk / maskb (and xt[0], loaded in the loop) gate the first
            # matmuls; wvo/biasc DMAs are emitted inside batch 0's body so
            # they don't delay the PE start on the FIFO DMA queue.
            wqk_sb = const_pool.tile([P, NC_, D], mdt, tag="wqk")
            for kc in range(NC_):
                nc.sync.dma_start(wqk_sb[:, kc, :],
                                  wqk_ap[kc * P:(kc + 1) * P, :])
            maskb_sb = const_pool.tile([P, BPC, NC_], f32, tag="maskb")
            nc.sync.dma_start(maskb_sb[:], maskb_ap[:])
            wvo_sb = const_pool.tile([P, NC_, D], mdt, tag="wvo")
            biasc_sb = const_pool.tile([P, NC_], f32, tag="biasc")
            # all-ones lhsT for the denominator reduction (f32r needs full
            # 128-column weights, so M=1 is padded to M=128: every output
            # row of the matmul is the same denominator row)
            ones_mat = const_pool.tile([P, P], mdt, tag="ones_mat")
            if mm_fast:
                # memset can't emit float32r directly; stage fp32 + cast-copy
                ones_f32 = const_pool.tile([P, P], f32, tag="ones_f32")
                nc.any.memset(ones_f32[:], 1.0)
                nc.vector.tensor_copy(ones_mat[:], ones_f32[:])
            else:
                nc.any.memset(ones_mat[:], 1.0)

            for b in range(BPC):
                # load X^T for this graph: [128, 4, 512] (i on partitions),
                # chunked so the first matmul only waits for chunk 0
                xt_sb = xt_pool.tile([P, NC_, G], mdt, tag="xt")
                for kc in range(NC_):
                    nc.sync.dma_start(xt_sb[:, kc, :],
                                      xt_ap[b, kc * P:(kc + 1) * P, :])

                # P1T[i', q] = Wqk^T X^T
                p1t_sb = p1t_pool.tile([P, NC_, G], mdt, tag="p1t")
                for mc in range(NC_):
                    ps = psum_pool.tile([P, G], f32, tag="big")
                    for kc in range(NC_):
                        nc.tensor.matmul(
                            ps[:],
                            wqk_sb[:, kc, mc * P:(mc + 1) * P],
                            xt_sb[:, kc, :],
                            start=(kc == 0), stop=(kc == NC_ - 1))
                    nc.vector.tensor_copy(p1t_sb[:, mc, :], ps[:])

                if b == 0:
                    for kc in range(NC_):
                        nc.sync.dma_start(wvo_sb[:, kc, :],
                                          wvo_ap[kc * P:(kc + 1) * P, :])
                    nc.sync.dma_start(biasc_sb[:], biasc_ap[:])

                # ST[k, q] = X P1 (scores transposed), then Pexp = exp(+bias)
                pexp_sb = pexp_pool.tile([P, NC_, G], mdt, tag="pexp")
                for mc in range(NC_):
                    ps = psum_pool.tile([P, G], f32, tag="big")
                    for kc in range(NC_):
                        nc.tensor.matmul(
                            ps[:],
                            xt_sb[:, kc, mc * P:(mc + 1) * P],
                            p1t_sb[:, kc, :],
                            start=(kc == 0), stop=(kc == NC_ - 1))
                    nc.scalar.activation(
                        pexp_sb[:, mc, :], ps[:],
                        mybir.ActivationFunctionType.Exp,
                        bias=maskb_sb[:, b, mc:mc + 1], scale=1.0)

                # V2[g, e] = X Wvo  (evacuated on ACT: Copy lives in every
                # activation table set, so no table switch against Exp)
                v2_sb = v2_pool.tile([P, NC_, D], mdt, tag="v2")
                for mc in range(NC_):
                    ps = psum_pool.tile([P, D], f32, tag="big")
                    for kc in range(NC_):
                        nc.tensor.matmul(
                            ps[:],
                            xt_sb[:, kc, mc * P:(mc + 1) * P],
                            wvo_sb[:, kc, :],
                            start=(kc == 0), stop=(kc == NC_ - 1))
                    nc.scalar.activation(
                        v2_sb[:, mc, :], ps[:],
                        mybir.ActivationFunctionType.Copy)

                # denominators: ones^T Pexp -> [128, 512], every row = den[q];
                # reciprocal of row 0, broadcast back to 128 partitions
                ps_row = psum_pool.tile([P, G], f32, tag="big")
                for kc in range(NC_):
                    nc.tensor.matmul(
                        ps_row[:],
                        ones_mat[:],
                        pexp_sb[:, kc, :],
                        start=(kc == 0), stop=(kc == NC_ - 1))
                recrow_sb = small_pool.tile([1, G], f32, tag="recrow")
                nc.vector.reciprocal_approx_fast(recrow_sb[:], ps_row[0:1, :])
                rb_sb = small_pool.tile([P, G], f32, tag="rb")
                nc.gpsimd.partition_broadcast(rb_sb[:], recrow_sb[:])

                # outT[e, q] = V2^T Pexp, normalized per-q + bias per-e
                out_sb = out_pool.tile([P, NC_, G], f32, tag="out")
                for mc in range(NC_):
                    ps = psum_pool.tile([P, G], f32, tag="big")
                    for kc in range(NC_):
                        nc.tensor.matmul(
                            ps[:],
                            v2_sb[:, kc, mc * P:(mc + 1) * P],
                            pexp_sb[:, kc, :],
                            start=(kc == 0), stop=(kc == NC_ - 1))
                    nc.vector.tensor_mul(out_sb[:, mc, :], ps[:], rb_sb[:])
                    nc.vector.tensor_scalar_add(
                        out_sb[:, mc, :], out_sb[:, mc, :],
                        biasc_sb[:, mc:mc + 1])
                    # store each 128-row chunk as soon as it is ready
                    nc.sync.dma_start(
                        yt_ap[b, mc * P:(mc + 1) * P, :],
                        out_sb[:, mc, :])

    nc.compile()
    return nc


def _get_nc(mm_fast: bool = True):
    key = ("nc", mm_fast)
    if key not in _CACHE:
        _CACHE[key] = _build_nc(mm_fast)
    return _CACHE[key]


def _host_prep(data, W_query, W_key, W_value, W_out_w, W_out_b, mask):
    data = np.asarray(data, dtype=np.float32)
    W_query = np.asarray(W_query, dtype=np.float32)
    W_key = np.asarray(W_key, dtype=np.float32)
    W_value = np.asarray(W_value, dtype=np.float32)
    W_out_w = np.asarray(W_out_w, dtype=np.float32)
    W_out_b = np.asarray(W_out_b, dtype=np.float32)
    mask = np.asarray(mask).astype(bool)

    norm = 1.0 / math.sqrt(D)
    wqk = (norm * (W_query.T.astype(np.float64) @ W_key.astype(np.float64))
           ).astype(np.float32)
    # V Wo^T = X (Wo Wv)^T: fold the value and output projections
    wvo = ((W_out_w.astype(np.float64) @ W_value.astype(np.float64)).T
           ).astype(np.float32)
    wvo = np.ascontiguousarray(wvo)
    # bias in [e]-on-partitions layout: biasc[p, mc] = b[mc*128 + p]
    biasc = np.ascontiguousarray(W_out_b.reshape(NC_, P).T)
    # [128, B, 4]: maskb[p, b, kc] = bias for key k = kc*128 + p of graph b
    maskb_full = np.where(mask, np.float32(MASK_BIAS), np.float32(0.0)) \
        .reshape(B, NC_, P).transpose(2, 0, 1).copy()
    # X^T per graph: [B, i, g]
    dataT = data.reshape(B, G, D).transpose(0, 2, 1)

    in_maps = []
    for c in range(N_CORES):
        b0 = c * BPC
        in_maps.append({
            "xt": np.ascontiguousarray(dataT[b0:b0 + BPC]),
            "wqk": wqk,
            "wvo": wvo,
            "biasc": biasc,
            "maskb": np.ascontiguousarray(maskb_full[:, b0:b0 + BPC, :]),
        })
    return in_maps


def kernel(data, W_query, W_key, W_value, W_out_w, W_out_b, mask,
           graph_size):
    from concourse.bass_utils import run_bass_kernel_spmd

    assert int(graph_size) == G
    in_maps = _host_prep(data, W_query, W_key, W_value, W_out_w, W_out_b,
                         mask)
    nc = _get_nc()
    res = run_bass_kernel_spmd(nc, in_maps, list(range(N_CORES)))
    yt = np.concatenate([res.results[c]["yt"] for c in range(N_CORES)],
                        axis=0)                      # [B, e, q]
    y = yt.transpose(0, 2, 1).reshape(B * G, D)      # [B*G, e]
    return np.ascontiguousarray(y, dtype=np.float32)


# revision 19
# speedup vs baseline: 1.5424x; 1.0107x over previous
"""Trainium2 Bass kernel for single-head (H=1) masked self-attention over
128 independent graphs of 512 nodes (d_model = 512).

Math (per graph b, X = data rows of b, all [512, 512]):
    S  = (1/sqrt(512)) * X Wq^T Wk X^T        (key-mask -> -inf cols)
    A  = softmax(S, axis=-1), masked cols zeroed
    out = A V Wo^T + b,  V = X Wv^T

Device strategy (data-parallel over batch, 16 graphs per NeuronCore).
Two host-side weight foldings cut the per-graph GEMM count from 6 to 4
(the provable minimum for this module):
    Wqk = norm * Wq^T @ Wk          ->  S   = X Wqk X^T      (2 GEMMs)
    Wvo = (Wo @ Wv)^T               ->  out = A (X Wvo) + b  (2 GEMMs)
Host supplies X^T per graph ([i, g]) so no on-device transposes are needed:
    P1T  = matmul(lhsT=Wqk,  rhs=XT)      # (X Wqk)^T        [i', q]
    ST   = matmul(lhsT=XT,   rhs=P1T)     # S^T              [k,  q]
    Pexp = exp(ST + mask_bias[k])         # ACT, bias -30000 on masked k
    V2   = matmul(lhsT=XT,   rhs=Wvo)     # X Wvo            [g,  e]
    den  = ones^T @ Pexp                  # softmax denominators
    outT = matmul(lhsT=V2,   rhs=Pexp) * (1/den)[q] + bias[e]   # [e, q]
The kernel writes the output transposed ([e, q] per graph); the host
transposes it back (pure layout work, like the input X^T prep).

Softmax runs in S^T layout (keys on partitions) so the mask bias is a
per-partition ACT bias and exp(-30000) = 0 reproduces the exact masked
zeros of the reference. Normalization uses an all-ones [128,128] lhsT
matmul for the denominators (f32r requires full 128-column weights), a
fast-approx reciprocal on DVE, and a GpSimd partition_broadcast so the
divide fuses into the output PSUM->SBUF evacuation.

All GEMMs run as float32r: single-pass PE mode, 4x the exact-fp32 rate,
~2e-4 relative error end-to-end.
"""

import math

import numpy as np

N_CORES = 8
B = 128          # graphs
G = 512          # nodes per graph
D = 512          # model dim
BPC = B // N_CORES   # graphs per core
P = 128          # SBUF partitions
NC_ = D // P     # 4 chunks of 128

MASK_BIAS = -30000.0

_CACHE: dict = {}


def _build_nc(mm_fast: bool):
    import concourse.tile as tile
    from concourse import bacc, mybir

    f32 = mybir.dt.float32
    mdt = mybir.dt.float32r if mm_fast else mybir.dt.float32

    nc = bacc.Bacc("TRN2", target_bir_lowering=False, debug=False,
                   num_devices=N_CORES)

    xt_ap = nc.dram_tensor("xt", [BPC, D, G], mdt, kind="ExternalInput").ap()
    wqk_ap = nc.dram_tensor("wqk", [D, D], mdt, kind="ExternalInput").ap()
    wvo_ap = nc.dram_tensor("wvo", [D, D], mdt, kind="ExternalInput").ap()
    biasc_ap = nc.dram_tensor("biasc", [P, NC_], f32,
                              kind="ExternalInput").ap()
    maskb_ap = nc.dram_tensor("maskb", [P, BPC, NC_], f32,
                              kind="ExternalInput").ap()
    # output transposed: yt[b] = out[b].T  ([e, q] layout)
    yt_ap = nc.dram_tensor("yt", [BPC, D, G], f32, kind="ExternalOutput").ap()

    with tile.TileContext(nc) as tc:
        with (
            tc.tile_pool(name="const", bufs=1) as const_pool,
            tc.tile_pool(name="xt", bufs=2) as xt_pool,
            tc.tile_pool(name="p1t", bufs=2) as p1t_pool,
            tc.tile_pool(name="pexp", bufs=2) as pexp_pool,
            tc.tile_pool(name="v2", bufs=2) as v2_pool,
            tc.tile_pool(name="outp", bufs=2) as out_pool,
            tc.tile_pool(name="small", bufs=2) as small_pool,
            tc.tile_pool(name="psum", bufs=8, space="PSUM") as psum_pool,
        ):
            # --- one-time constants ---
            # Only wqk / maskb (and xt[0], loaded in the loop) gate the first
            # matmuls; wvo/biasc DMAs are emitted inside batch 0's body so
            # they don't delay the PE start on the FIFO DMA queue.
            wqk_sb = const_pool.tile([P, NC_, D], mdt, tag="wqk")
            xt0_sb = xt_pool.tile([P, NC_, G], mdt, tag="xt")
            for kc in range(NC_):
                # interleave so matmul (mc=0, kc=0) can start after the
                # first two transfers instead of after all of wqk
                nc.sync.dma_start(wqk_sb[:, kc, :],
                                  wqk_ap[kc * P:(kc + 1) * P, :])
                nc.sync.dma_start(xt0_sb[:, kc, :],
                                  xt_ap[0, kc * P:(kc + 1) * P, :])
            maskb_sb = const_pool.tile([P, BPC, NC_], f32, tag="maskb")
            nc.sync.dma_start(maskb_sb[:], maskb_ap[:])
            wvo_sb = const_pool.tile([P, NC_, D], mdt, tag="wvo")
            biasc_sb = const_pool.tile([P, NC_], f32, tag="biasc")
            # all-ones lhsT for the denominator reduction (f32r needs full
            # 128-column weights, so M=1 is padded to M=128: every output
            # row of the matmul is the same denominator row)
            ones_mat = const_pool.tile([P, P], mdt, tag="ones_mat")
            if mm_fast:
                # memset can't emit float32r directly; stage fp32 + cast-copy
                ones_f32 = const_pool.tile([P, P], f32, tag="ones_f32")
                nc.any.memset(ones_f32[:], 1.0)
                nc.vector.tensor_copy(ones_mat[:], ones_f32[:])
            else:
                nc.any.memset(ones_mat[:], 1.0)

            for b in range(BPC):
                # load X^T for this graph: [128, 4, 512] (i on partitions),
                # chunked so the first matmul only waits for chunk 0
                # (batch 0's load is interleaved with wqk above)
                if b == 0:
                    xt_sb = xt0_sb
                else:
                    xt_sb = xt_pool.tile([P, NC_, G], mdt, tag="xt")
                    for kc in range(NC_):
                        nc.sync.dma_start(xt_sb[:, kc, :],
                                          xt_ap[b, kc * P:(kc + 1) * P, :])

                # P1T[i', q] = Wqk^T X^T
                p1t_sb = p1t_pool.tile([P, NC_, G], mdt, tag="p1t")
                for mc in range(NC_):
                    ps = psum_pool.tile([P, G], f32, tag="big")
                    for kc in range(NC_):
                        nc.tensor.matmul(
                            ps[:],
                            wqk_sb[:, kc, mc * P:(mc + 1) * P],
                            xt_sb[:, kc, :],
                            start=(kc == 0), stop=(kc == NC_ - 1))
                    nc.vector.tensor_copy(p1t_sb[:, mc, :], ps[:])

                if b == 0:
                    for kc in range(NC_):
                        nc.sync.dma_start(wvo_sb[:, kc, :],
                                          wvo_ap[kc * P:(kc + 1) * P, :])
                    nc.sync.dma_start(biasc_sb[:], biasc_ap[:])

                # ST[k, q] = X P1 (scores transposed), then Pexp = exp(+bias)
                pexp_sb = pexp_pool.tile([P, NC_, G], mdt, tag="pexp")
                for mc in range(NC_):
                    ps = psum_pool.tile([P, G], f32, tag="big")
                    for kc in range(NC_):
                        nc.tensor.matmul(
                            ps[:],
                            xt_sb[:, kc, mc * P:(mc + 1) * P],
                            p1t_sb[:, kc, :],
                            start=(kc == 0), stop=(kc == NC_ - 1))
                    nc.scalar.activation(
                        pexp_sb[:, mc, :], ps[:],
                        mybir.ActivationFunctionType.Exp,
                        bias=maskb_sb[:, b, mc:mc + 1], scale=1.0)

                # V2[g, e] = X Wvo  (evacuated on ACT: Copy lives in every
                # activation table set, so no table switch against Exp)
                v2_sb = v2_pool.tile([P, NC_, D], mdt, tag="v2")
                for mc in range(NC_):
                    ps = psum_pool.tile([P, D], f32, tag="big")
                    for kc in range(NC_):
                        nc.tensor.matmul(
                            ps[:],
                            xt_sb[:, kc, mc * P:(mc + 1) * P],
                            wvo_sb[:, kc, :],
                            start=(kc == 0), stop=(kc == NC_ - 1))
                    nc.scalar.activation(
                        v2_sb[:, mc, :], ps[:],
                        mybir.ActivationFunctionType.Copy)

                # denominators: ones^T Pexp -> [128, 512], every row = den[q];
                # reciprocal of row 0, broadcast back to 128 partitions
                ps_row = psum_pool.tile([P, G], f32, tag="big")
                for kc in range(NC_):
                    nc.tensor.matmul(
                        ps_row[:],
                        ones_mat[:],
                        pexp_sb[:, kc, :],
                        start=(kc == 0), stop=(kc == NC_ - 1))
                recrow_sb = small_pool.tile([1, G], f32, tag="recrow")
                nc.vector.reciprocal_approx_fast(recrow_sb[:], ps_row[0:1, :])
                rb_sb = small_pool.tile([P, G], f32, tag="rb")
                nc.gpsimd.partition_broadcast(rb_sb[:], recrow_sb[:])

                # outT[e, q] = V2^T Pexp, normalized per-q + bias per-e
                out_sb = out_pool.tile([P, NC_, G], f32, tag="out")
                for mc in range(NC_):
                    ps = psum_pool.tile([P, G], f32, tag="big")
                    for kc in range(NC_):
                        nc.tensor.matmul(
                            ps[:],
                            v2_sb[:, kc, mc * P:(mc + 1) * P],
                            pexp_sb[:, kc, :],
                            start=(kc == 0), stop=(kc == NC_ - 1))
                    nc.vector.tensor_mul(out_sb[:, mc, :], ps[:], rb_sb[:])
                    nc.vector.tensor_scalar_add(
                        out_sb[:, mc, :], out_sb[:, mc, :],
                        biasc_sb[:, mc:mc + 1])
                    # store each 128-row chunk as soon as it is ready
                    nc.sync.dma_start(
                        yt_ap[b, mc * P:(mc + 1) * P, :],
                        out_sb[:, mc, :])

    nc.compile()
    return nc


def _get_nc(mm_fast: bool = True):
    key = ("nc", mm_fast)
    if key not in _CACHE:
        _CACHE[key] = _build_nc(mm_fast)
    return _CACHE[key]


def _host_prep(data, W_query, W_key, W_value, W_out_w, W_out_b, mask):
    data = np.asarray(data, dtype=np.float32)
    W_query = np.asarray(W_query, dtype=np.float32)
    W_key = np.asarray(W_key, dtype=np.float32)
    W_value = np.asarray(W_value, dtype=np.float32)
    W_out_w = np.asarray(W_out_w, dtype=np.float32)
    W_out_b = np.asarray(W_out_b, dtype=np.float32)
    mask = np.asarray(mask).astype(bool)

    norm = 1.0 / math.sqrt(D)
    wqk = (norm * (W_query.T.astype(np.float64) @ W_key.astype(np.float64))
           ).astype(np.float32)
    # V Wo^T = X (Wo Wv)^T: fold the value and output projections
    wvo = ((W_out_w.astype(np.float64) @ W_value.astype(np.float64)).T
           ).astype(np.float32)
    wvo = np.ascontiguousarray(wvo)
    # bias in [e]-on-partitions layout: biasc[p, mc] = b[mc*128 + p]
    biasc = np.ascontiguousarray(W_out_b.reshape(NC_, P).T)
    # [128, B, 4]: maskb[p, b, kc] = bias for key k = kc*128 + p of graph b
    maskb_full = np.where(mask, np.float32(MASK_BIAS), np.float32(0.0)) \
        .reshape(B, NC_, P).transpose(2, 0, 1).copy()
    # X^T per graph: [B, i, g]
    dataT = data.reshape(B, G, D).transpose(0, 2, 1)

    in_maps = []
    for c in range(N_CORES):
        b0 = c * BPC
        in_maps.append({
            "xt": np.ascontiguousarray(dataT[b0:b0 + BPC]),
            "wqk": wqk,
            "wvo": wvo,
            "biasc": biasc,
            "maskb": np.ascontiguousarray(maskb_full[:, b0:b0 + BPC, :]),
        })
    return in_maps


def kernel(data, W_query, W_key, W_value, W_out_w, W_out_b, mask,
           graph_size):
    import time

    from concourse.bass_utils import run_bass_kernel_spmd

    assert int(graph_size) == G
    in_maps = _host_prep(data, W_query, W_key, W_value, W_out_w, W_out_b,
                         mask)
    nc = _get_nc()
    # the axon-tunneled devices occasionally report a transient
    # NRT_EXEC_UNIT_UNRECOVERABLE; a fresh attempt recovers
    last_err = None
    for attempt in range(3):
        try:
            res = run_bass_kernel_spmd(nc, in_maps, list(range(N_CORES)))
            break
        except Exception as e:  # noqa: BLE001
            last_err = e
            time.sleep(5.0 * (attempt + 1))
    else:
        raise last_err
    yt = np.concatenate([res.results[c]["yt"] for c in range(N_CORES)],
                        axis=0)                      # [B, e, q]
    y = yt.transpose(0, 2, 1).reshape(B * G, D)      # [B*G, e]
    return np.ascontiguousarray(y, dtype=np.float32)
